# revision 23
# baseline (speedup 1.0000x reference)
"""GRAM-style GNN message passing kernel for 8 Trainium2 NeuronCores.

Model (see reference):
  1. Ontology attention: gather leaf/ancestor embedding rows, small MLP
     (tanh) -> softmax over L=5 ancestors -> emb [V, E].
  2. x_emb = tanh(x @ emb)          [T, B, E]   (the big GEMM)
  3. 50-step GRU scan over T        [T, B, H]
  4. out = softmax(hidden @ W_out + b_out) * mask

Sharding: phase 1 is sharded over 1024-aligned v-blocks (core c computes
rows [1024c, 1024c+1024) of emb; core 7's tail is padding), followed by an
AllGather of the fp16 table.  Phases 2-4 are data-parallel over batch B
(16 of 128 per core).

Performance structure (vs the naive version):
  - all of x^T (12.8 MB fp16) is prefetched into SBUF at t=0 on the sync
    DMA queue, hiding the HBM stream under phases 1-2;
  - phase A is pipelined per 128-row v-tile with vt-major table layouts;
  - the GRU input-gate GEMM gx = W_ih^T @ x_emb is computed once for all
    50 steps; the r/z parts are pre-filled into PSUM banks so the per-step
    hh matmul accumulates on top and the sigmoid reads PSUM directly;
  - the per-step chain is 3 matmuls + 3 activations + 4 DVE ops (+2 on
    GpSimd off the critical path);
  - the output softmax (exp) is deferred past the scan so the scalar
    engine never swaps activation tables inside the loop.
"""

import numpy as np

import concourse.bass as bass
import concourse.mybir as mybir
import concourse.tile as tile
from concourse import bacc
from concourse.bass_utils import run_bass_kernel_spmd

FP32 = mybir.dt.float32
FP16 = mybir.dt.float16

AF = mybir.ActivationFunctionType
ALU = mybir.AluOpType
AX = mybir.AxisListType

T, B, V = 50, 128, 8000
A = 728
NEMB = V + A          # 8728
E, H, ATT, C, L = 128, 128, 100, 283, 5
NCORES = 8
BL = B // NCORES      # 16 batch per core
M = T * BL            # 800 tokens per core (m = t*BL + b)
VSH = 1024            # aligned v-shard rows per core
VFULL = VSH * NCORES  # 8192 (padded V)
NIDX = L * VSH        # 5120 gather indices per table
NVT = VSH // 128      # 8 v-tiles per shard
VTC = L * 128         # 640 cols per v-tile in the vt-major tables
KT = (V + 127) // 128 # 63 contraction tiles for the big GEMM
MT = (M + 127) // 128 # 7 m-tiles for the output phase
NH = M // 2           # 400-col psum halves

_CACHE = {}


def _build_nc():
    nc = bacc.Bacc(
        "TRN2",
        target_bir_lowering=False,
        debug=False,
        num_devices=NCORES,
    )

    # ---- DRAM I/O ----
    xt = nc.dram_tensor("xt", [V, M], FP16, kind="ExternalInput").ap()
    leT_d = nc.dram_tensor("leT_d", [E, NIDX], FP16, kind="ExternalInput").ap()
    anT_d = nc.dram_tensor("anT_d", [E, NIDX], FP16, kind="ExternalInput").ap()
    an3_d = nc.dram_tensor("an3_d", [128, NIDX], FP16, kind="ExternalInput").ap()
    # all small weights host-packed into two tensors -> two DMAs
    pack16 = nc.dram_tensor("pack16", [128, 1252], FP16, kind="ExternalInput").ap()
    pack32 = nc.dram_tensor("pack32", [128, 295], FP32, kind="ExternalInput").ap()
    out_d = nc.dram_tensor("out", [M, C], FP32, kind="ExternalOutput").ap()

    emb_shard = nc.dram_tensor("emb_shard", [VSH, E], FP16).ap()
    emb_full = nc.dram_tensor("emb_full", [VFULL, E], FP16, addr_space="Shared").ap()

    with tile.TileContext(nc) as tc:
        _emit(nc, tc, locals())
    nc.compile()
    return nc


def _emit(nc, tc, t):
    xt = t["xt"]
    leT_d, anT_d, an3_d = t["leT_d"], t["anT_d"], t["an3_d"]
    pack16, pack32, out_d = t["pack16"], t["pack32"], t["out_d"]
    emb_shard, emb_full = t["emb_shard"], t["emb_full"]

    with tc.tile_pool(name="const", bufs=1) as constp:
        # ---- two packed const DMAs first on the sync queue ----
        pack16_sb = constp.tile([128, 1252], FP16)
        nc.sync.dma_start(pack16_sb[:], pack16[:, :])
        pack32_sb = constp.tile([128, 295], FP32)
        nc.sync.dma_start(pack32_sb[:], pack32[:, :])
        watt_top_sb = pack16_sb[:, 0:100]
        watt_bot_sb = pack16_sb[:, 100:200]
        vatt_sb = pack16_sb[0:100, 200:201]
        wih_sb = pack16_sb[:, 201:585]
        whh_sb = pack16_sb[:, 585:969]
        wout_sb = pack16_sb[:, 969:1252]
        batt_sb = pack32_sb[0:100, 0:1]
        b_r_sb = pack32_sb[:, 1:2]
        b_z_sb = pack32_sb[:, 2:3]
        b_in_sb = pack32_sb[:, 3:4]
        b_hn_sb = pack32_sb[:, 4:5]
        masks_sb = pack32_sb[:, 5:12]
        bout_sb = pack32_sb[0:1, 12:295]

        # ---- phase A gather tables, interleaved in 2-vt chunks ----
        leT = constp.tile([128, NIDX], FP16)
        anT = constp.tile([128, NIDX], FP16)
        an3 = constp.tile([128, NIDX], FP16)
        CW = 2 * VTC  # 1280 cols = 2 v-tiles
        for g in range(NIDX // CW):
            sl = bass.ts(g, CW)
            nc.sync.dma_start(leT[:, sl], leT_d[:, sl])
            nc.sync.dma_start(anT[:, sl], anT_d[:, sl])
            nc.sync.dma_start(an3[:, sl], an3_d[:, sl])
        an3v = an3[:].rearrange("p (i e) -> p i e", e=E)

        # ---- x^T prefetch into SBUF.  Groups 0-5 go on the sync queue at
        # t=0 (finishing as phase A ends); groups 6+ are issued later on the
        # scalar queue BEHIND the post-collective emb reloads, so the
        # AllGather's own DMAs aren't starved by the x stream. ----
        xt_sb = constp.tile([128, KT * M], FP16)
        KG = 8  # k-tiles per DMA descriptor
        NEARLY = 5
        for g in range(NEARLY):
            nc.sync.dma_start(
                xt_sb[:, g * KG * M : (g + 1) * KG * M].rearrange(
                    "p (c m) -> p c m", m=M
                ),
                xt[g * KG * 128 : (g + 1) * KG * 128, :].rearrange(
                    "(c p) m -> p c m", p=128
                ),
            )

        def _xt_late_dmas():
            full_rows = (V // (KG * 128)) * KG * 128   # 7168
            for g in range(NEARLY, full_rows // (KG * 128)):
                nc.scalar.dma_start(
                    xt_sb[:, g * KG * M : (g + 1) * KG * M].rearrange(
                        "p (c m) -> p c m", m=M
                    ),
                    xt[g * KG * 128 : (g + 1) * KG * 128, :].rearrange(
                        "(c p) m -> p c m", p=128
                    ),
                )
            rest0 = full_rows                           # 7168
            rest_full = ((V - rest0) // 128) * 128      # 768
            nc.scalar.dma_start(
                xt_sb[:, rest0 // 128 * M : (rest0 + rest_full) // 128 * M].rearrange(
                    "p (c m) -> p c m", m=M
                ),
                xt[rest0 : rest0 + rest_full, :].rearrange("(c p) m -> p c m", p=128),
            )
            tail0 = rest0 + rest_full                   # 7936
            nc.scalar.dma_start(
                xt_sb[0 : V - tail0, tail0 // 128 * M : tail0 // 128 * M + M],
                xt[tail0:V, :],
            )

        # =====================================================================
        # Phase A: ontology attention, pipelined per v-tile -> emb_own/shard
        # =====================================================================
        emb_own = constp.tile([128, NVT * E], FP16)
        with (
            tc.tile_pool(name="pa_sb", bufs=2) as pa_sb,
            tc.tile_pool(name="mlp_ps", bufs=2, space="PSUM") as mlp_ps,
            tc.tile_pool(name="pre_ps", bufs=2, space="PSUM") as pre_ps,
        ):
            mlp_sb = constp.tile([ATT, NIDX], FP16)
            MLPC = VTC // 2  # 320-col psum chunks
            for vt in range(NVT):
                base = vt * VTC
                for hf in range(2):
                    sl = slice(base + hf * MLPC, base + (hf + 1) * MLPC)
                    ps = mlp_ps.tile([ATT, MLPC], FP32, tag="mlp")
                    nc.tensor.matmul(ps[:], watt_top_sb[:], leT[:, sl], start=True, stop=False)
                    nc.tensor.matmul(ps[:], watt_bot_sb[:], anT[:, sl], start=False, stop=True)
                    nc.scalar.activation(mlp_sb[:, sl], ps[:], AF.Tanh, bias=batt_sb[:, 0:1])

                psp = pre_ps.tile([128, 8], FP32, tag="pre")
                for l in range(L):
                    nc.tensor.matmul(
                        psp[:, l : l + 1],
                        mlp_sb[:, base + l * 128 : base + (l + 1) * 128],
                        vatt_sb[:],
                        start=True,
                        stop=True,
                        skip_group_check=True,
                    )
                # |pre| <~ 50, so exp is safe in fp32 without max-subtraction
                att = pa_sb.tile([128, L], FP32, tag="att")
                asum = pa_sb.tile([128, 1], FP32, tag="asum")
                nc.scalar.activation(
                    att[:], psp[:, 0:L], AF.Exp, accum_out=asum[:, 0:1]
                )
                arec = pa_sb.tile([128, 1], FP32, tag="arec")
                nc.vector.reciprocal(arec[:], asum[:])
                attn = pa_sb.tile([128, L], FP32, tag="attn")
                nc.vector.tensor_scalar(
                    attn[:], att[:], arec[:, 0:1], None, op0=ALU.mult
                )
                acc = pa_sb.tile([128, E], FP32, tag="acc")
                nc.vector.tensor_scalar(
                    acc[:], an3v[:, vt * L, :], attn[:, 0:1], None, op0=ALU.mult
                )
                for l in range(1, L - 1):
                    nc.vector.scalar_tensor_tensor(
                        acc[:],
                        an3v[:, vt * L + l, :],
                        attn[:, l : l + 1],
                        acc[:],
                        op0=ALU.mult,
                        op1=ALU.add,
                    )
                embt = emb_own[:, vt * E : (vt + 1) * E]
                nc.vector.scalar_tensor_tensor(
                    embt,
                    an3v[:, vt * L + L - 1, :],
                    attn[:, L - 1 : L],
                    acc[:],
                    op0=ALU.mult,
                    op1=ALU.add,
                )
                nc.scalar.dma_start(
                    emb_shard[vt * 128 : (vt + 1) * 128, :], embt
                )

        # AllGather the fp16 embedding table across the 8 cores.
        nc.gpsimd.collective_compute(
            "AllGather",
            ALU.bypass,
            replica_groups=[list(range(NCORES))],
            ins=[emb_shard[:, :]],
            outs=[emb_full[:, :]],
        )

        # =====================================================================
        # Phase B: x_emb^T = tanh(emb^T @ x^T)   [E, M] fp16
        # =====================================================================
        xemb = constp.tile([E, M], FP16)
        with (
            tc.tile_pool(name="embp", bufs=8) as embp,
            tc.tile_pool(name="pb_ps", bufs=1, space="PSUM") as pb_ps,
        ):
            ps_a = pb_ps.tile([128, NH], FP32, tag="ps_a")
            ps_b = pb_ps.tile([128, NH], FP32, tag="ps_b")
            embgs = []
            for g in range(NCORES):
                ntile = min(KT - g * 8, 8)
                embg = embp.tile([128, 8 * E], FP16, name=f"embg{g}", tag="embg")
                nc.scalar.dma_start(
                    embg[:, 0 : ntile * E].rearrange("p (c e) -> p c e", e=E),
                    emb_full[g * VSH : g * VSH + ntile * 128, :].rearrange(
                        "(c p) e -> p c e", p=128
                    ),
                )
                embgs.append(embg)
            # late x groups ride the scalar queue behind the reloads, i.e.
            # gated by the collective via in-order queue processing
            _xt_late_dmas()
            for g in range(NCORES):
                ntile = min(KT - g * 8, 8)
                embg = embgs[g]
                for i in range(ntile):
                    kt = g * 8 + i
                    kp = min(128, V - kt * 128)
                    st, sp = kt == 0, kt == KT - 1
                    nc.tensor.matmul(
                        ps_a[:],
                        embg[0:kp, i * E : (i + 1) * E],
                        xt_sb[0:kp, kt * M : kt * M + NH],
                        start=st,
                        stop=sp,
                    )
                    nc.tensor.matmul(
                        ps_b[:],
                        embg[0:kp, i * E : (i + 1) * E],
                        xt_sb[0:kp, kt * M + NH : kt * M + M],
                        start=st,
                        stop=sp,
                    )
            nc.scalar.activation(xemb[:, 0:NH], ps_a[:], AF.Tanh)
            nc.scalar.activation(xemb[:, NH:M], ps_b[:], AF.Tanh)

        # =====================================================================
        # Phase C+D: gx pre-fill, GRU scan, deferred output softmax
        # =====================================================================
        hid = []
        for k in range(MT):
            mw = min(128, M - k * 128)
            hid.append(constp.tile([H, mw], FP16, name=f"hid{k}", tag=f"hid{k}"))
        lgt = constp.tile([128, MT * C], FP32)
        h0 = constp.tile([H, BL], FP16)
        nc.vector.memset(h0[:], 0.0)
        gxn_sb = constp.tile([H, M], FP32)
        bb_sb = constp.tile([128, C], FP32)

        with tc.tile_pool(name="gx_ps", bufs=1, space="PSUM") as gx_ps:
            gr = [gx_ps.tile([H, NH], FP32, name=f"gr{i}", tag=f"gr{i}") for i in range(2)]
            gz = [gx_ps.tile([H, NH], FP32, name=f"gz{i}", tag=f"gz{i}") for i in range(2)]
            for i in range(2):
                sl = bass.ts(i, NH)
                nc.tensor.matmul(
                    gr[i][:], wih_sb[:, 0:H], xemb[:, sl], start=True, stop=False,
                    skip_group_check=True,
                )
                nc.tensor.matmul(
                    gz[i][:], wih_sb[:, H : 2 * H], xemb[:, sl], start=True, stop=False,
                    skip_group_check=True,
                )
            with tc.tile_pool(name="gn_ps", bufs=1, space="PSUM") as gn_ps:
                for i in range(2):
                    sl = bass.ts(i, NH)
                    gn = gn_ps.tile([H, NH], FP32, tag=f"gn{i}")
                    nc.tensor.matmul(
                        gn[:], wih_sb[:, 2 * H : 3 * H], xemb[:, sl],
                        start=True, stop=True,
                    )
                    nc.scalar.activation(gxn_sb[:, sl], gn[:], AF.Copy)

            with (
                tc.tile_pool(name="gru_ps", bufs=2, space="PSUM") as gru_ps,
                tc.tile_pool(name="gru_sb", bufs=3) as gru_sb,
                tc.tile_pool(name="o_ps", bufs=1, space="PSUM") as o_ps,
                tc.tile_pool(name="o_sb", bufs=2) as o_sb,
            ):
                # broadcast b_out across partitions with a rank-1 matmul
                ones_sb = constp.tile([1, 128], FP32)
                nc.vector.memset(ones_sb[:], 1.0)
                psbb = o_ps.tile([128, C], FP32, tag="o")
                nc.tensor.matmul(psbb[:], ones_sb[:], bout_sb[:], start=True, stop=True)
                nc.vector.tensor_copy(bb_sb[:], psbb[:])

                for st_ in range(T):
                    half, off = st_ // 25, (st_ % 25) * BL
                    grb = gr[half][:, off : off + BL]
                    gzb = gz[half][:, off : off + BL]
                    hprev = h0[:] if st_ == 0 else hid[(st_ - 1) // 8][
                        :, ((st_ - 1) % 8) * BL : ((st_ - 1) % 8) * BL + BL
                    ]
                    # hh matmuls; r/z accumulate onto the pre-filled gx banks
                    nc.tensor.matmul(grb, whh_sb[:, 0:H], hprev, start=False,
                                     stop=True, skip_group_check=True)
                    nc.tensor.matmul(gzb, whh_sb[:, H : 2 * H], hprev, start=False,
                                     stop=True, skip_group_check=True)
                    ps_n = gru_ps.tile([H, BL], FP32, tag="psn")
                    nc.tensor.matmul(ps_n[:], whh_sb[:, 2 * H : 3 * H], hprev,
                                     start=True, stop=True)

                    r = gru_sb.tile([H, BL], FP32, tag="r")
                    nc.scalar.activation(r[:], grb, AF.Sigmoid, bias=b_r_sb[:, 0:1])
                    z = gru_sb.tile([H, BL], FP32, tag="z")
                    nc.scalar.activation(z[:], gzb, AF.Sigmoid, bias=b_z_sb[:, 0:1])
                    # off-critical-path, early in DVE queue order so h' need
                    # not wait on a cross-engine semaphore
                    zc = gru_sb.tile([H, BL], FP32, tag="zc")
                    nc.vector.tensor_scalar(zc[:], z[:], -1.0, 1.0,
                                            op0=ALU.mult, op1=ALU.add)
                    e2 = gru_sb.tile([H, BL], FP32, tag="e2")
                    nc.vector.tensor_mul(e2[:], z[:], hprev)
                    # n = tanh(gxn + b_in + r*(hn + b_hn))
                    t1 = gru_sb.tile([H, BL], FP32, tag="t1")
                    nc.vector.scalar_tensor_tensor(
                        t1[:], ps_n[:], b_hn_sb[:, 0:1], r[:],
                        op0=ALU.add, op1=ALU.mult,
                    )
                    t2 = gru_sb.tile([H, BL], FP32, tag="t2")
                    nc.vector.tensor_add(t2[:], t1[:], gxn_sb[:, st_ * BL : (st_ + 1) * BL])
                    n = gru_sb.tile([H, BL], FP32, tag="n")
                    nc.scalar.activation(n[:], t2[:], AF.Tanh, bias=b_in_sb[:, 0:1])
                    e1 = gru_sb.tile([H, BL], FP32, tag="e1")
                    nc.vector.tensor_mul(e1[:], zc[:], n[:])
                    hcur = hid[st_ // 8][:, (st_ % 8) * BL : (st_ % 8) * BL + BL]
                    nc.vector.tensor_add(hcur, e1[:], e2[:])

                    # logits matmul per completed m-tile; bias folded via the
                    # DVE add into the single lgt buffer (still no exp here,
                    # so the ACT engine keeps the sigmoid/tanh table)
                    if (st_ + 1) % 8 == 0 or st_ == T - 1:
                        k = st_ // 8
                        mw = min(128, M - k * 128)
                        pso = o_ps.tile([128, C], FP32, tag="o")
                        nc.tensor.matmul(
                            pso[0:mw, :], hid[k][:, 0:mw], wout_sb[:],
                            start=True, stop=True,
                        )
                        nc.vector.tensor_add(
                            lgt[0:mw, k * C : (k + 1) * C], pso[0:mw, :],
                            bb_sb[0:mw, :],
                        )

                # deferred softmax tail: one exp over all tiles (single act
                # table swap), then per-tile normalize.  |logits| <= ~13 so
                # exp without max-subtraction is safe in fp32.
                ex = constp.tile([128, MT * C], FP32)
                nc.scalar.activation(ex[:], lgt[:], AF.Exp)
                sums = o_sb.tile([128, MT], FP32, tag="sums")
                nc.vector.tensor_reduce(
                    sums[:], ex[:].rearrange("p (k c) -> p k c", c=C), AX.X, ALU.add
                )
                rec = o_sb.tile([128, MT], FP32, tag="rec")
                nc.vector.reciprocal(rec[:], sums[:])
                ob = constp.tile([128, MT * C], FP32)
                nft = M // 128  # 6 full tiles + a 32-row tail
                for k in range(MT):
                    mw = min(128, M - k * 128)
                    nc.vector.tensor_scalar(
                        ob[0:mw, k * C : (k + 1) * C],
                        ex[0:mw, k * C : (k + 1) * C],
                        rec[0:mw, k : k + 1],
                        masks_sb[0:mw, k : k + 1],
                        op0=ALU.mult, op1=ALU.mult,
                    )
                    # stream the output out as soon as each pair is ready
                    if k % 2 == 1:
                        nc.sync.dma_start(
                            out_d[(k - 1) * 128 : (k + 1) * 128, :].rearrange(
                                "(k2 p) c -> p k2 c", p=128
                            ),
                            ob[:, (k - 1) * C : (k + 1) * C].rearrange(
                                "p (k2 c) -> p k2 c", c=C
                            ),
                        )
                nc.sync.dma_start(
                    out_d[nft * 128 : M, :],
                    ob[0 : M - nft * 128, nft * C : MT * C],
                )


def _prep_inputs(inputs):
    x = np.asarray(inputs["x"], np.float32)
    mask = np.asarray(inputs["mask"], np.float32)
    W_emb = np.asarray(inputs["W_emb"], np.float32)
    W_att = np.asarray(inputs["W_attention"], np.float32)
    b_att = np.asarray(inputs["b_attention"], np.float32)
    v_att = np.asarray(inputs["v_attention"], np.float32)
    w_ih = np.asarray(inputs["gru_w_ih"], np.float32)
    w_hh = np.asarray(inputs["gru_w_hh"], np.float32)
    b_ih = np.asarray(inputs["gru_b_ih"], np.float32)
    b_hh = np.asarray(inputs["gru_b_hh"], np.float32)
    W_out = np.asarray(inputs["W_output"], np.float32)
    b_out = np.asarray(inputs["b_output"], np.float32)
    leaves = np.asarray(inputs["leaves"])
    ancestors = np.asarray(inputs["ancestors"])

    p16 = np.zeros((128, 1252), np.float16)
    p16[:, 0:100] = W_att[:E, :]
    p16[:, 100:200] = W_att[E:, :]
    p16[0:ATT, 200] = v_att
    p16[:, 201:585] = w_ih.T
    p16[:, 585:969] = w_hh.T
    p16[:, 969:1252] = W_out
    p32 = np.zeros((128, 295), np.float32)
    p32[0:ATT, 0] = b_att
    p32[:, 1] = b_ih[0:H] + b_hh[0:H]
    p32[:, 2] = b_ih[H : 2 * H] + b_hh[H : 2 * H]
    p32[:, 3] = b_ih[2 * H : 3 * H]
    p32[:, 4] = b_hh[2 * H : 3 * H]
    p32[0, 12:295] = b_out
    shared = {"pack16": p16}

    W16 = W_emb.astype(np.float16)
    in_maps = []
    for c in range(NCORES):
        m = dict(shared)
        xc = x[:, c * BL : (c + 1) * BL, :].reshape(M, V)
        m["xt"] = np.ascontiguousarray(xc.T).astype(np.float16)
        # masks_sb[p, k] = mask[k*128 + p] for this core's token stream
        p32c = p32.copy()
        mcol = mask[:, c * BL : (c + 1) * BL].reshape(M)
        mpad = np.zeros(MT * 128, np.float32)
        mpad[0:M] = mcol
        p32c[:, 5:12] = mpad.reshape(MT, 128).T
        m["pack32"] = p32c
        lo = c * VSH
        hi = min(V, lo + VSH)
        lv = np.zeros((VSH, L), np.int64)
        lv[0 : hi - lo] = leaves[lo:hi, :]
        av = np.zeros((VSH, L), np.int64)
        av[0 : hi - lo] = ancestors[lo:hi, :]
        # vt-major index layout: col j = vt*(L*128) + l*128 + p, v = vt*128+p
        le_idx = lv.reshape(NVT, 128, L).transpose(0, 2, 1).reshape(-1)
        an_idx = av.reshape(NVT, 128, L).transpose(0, 2, 1).reshape(-1)
        le_rows = W16[le_idx, :]                       # [NIDX, E]
        an_rows = W16[an_idx, :]
        m["leT_d"] = np.ascontiguousarray(le_rows.T)   # [E, NIDX]
        m["anT_d"] = np.ascontiguousarray(an_rows.T)
        # an3[p, (vt*L+l)*E + e] = an_rows[(vt, l, p), e]
        m["an3_d"] = np.ascontiguousarray(
            an_rows.reshape(NVT, L, 128, E).transpose(2, 0, 1, 3).reshape(128, NIDX)
        )
        in_maps.append(m)
    return in_maps


def kernel(**inputs):
    if "nc" not in _CACHE:
        _CACHE["nc"] = _build_nc()
    nc = _CACHE["nc"]
    in_maps = _prep_inputs(inputs)
    res = run_bass_kernel_spmd(nc, in_maps, list(range(NCORES)))
    out = np.empty((T, B, C), np.float32)
    for c in range(NCORES):
        out[:, c * BL : (c + 1) * BL, :] = res.results[c]["out"].reshape(T, BL, C)
    return out
